# revision 1
# baseline (speedup 1.0000x reference)
"""Trainium2 Bass kernel for stacked-LSTM + attention + dense head.

Model (per reference):
  3x LSTM(H=512, return_sequences) with inference BatchNorm between layers,
  attention pooling over time, then Dense(128)+BN+Dense(64)+Dense(5).
  B=128, T=512, D=128, H=512, fp32.

Strategy: data-parallel over batch (16 rows/core on 8 cores). Per core a
3-layer *wavefront*: every slot s advances layer0 at t=s, layer1 at
t=s-10, layer2 at t=s-20. The three recurrent matmuls run concurrently in
PE column groups 0..2 (tile_position), and column group 3 computes the
next layers' input projections (xz = h @ W, two steps per slot, bias
added at eviction). Gate activations are stacked across the three layers
(partition groups 0-15/32-47/64-79) so ACT/DVE cost is paid once, not
three times. h is re-transposed each step via DVE StreamTranspose (32x32
blocks) + a partition-scatter DMA. Recurrent/projection matmuls use bf16
operands with fp32 PSUM accumulation; the layer-0 input projection and
attention pooling use f32r (full-rate TF32-like) matmuls.

Self-contained: hardcodes shapes; no reads of reference.py/spec.json.
"""

import functools

import numpy as np

B, T, D, H = 128, 512, 128, 512
NC = 8
BL = B // NC          # batch rows per core
G4 = 4 * H            # gate width 2048
EPS = 1e-3
P = 128
LAG1, LAG2 = 10, 20   # wavefront lags of layers 1 and 2
RING = 8              # xz ring slots
HR = 16               # hT time-ring length

# column permutation: keras gate order [i|f|g|o] -> kernel order [i|f|o|g]
_PERM = np.concatenate([
    np.arange(0, 512), np.arange(512, 1024),
    np.arange(1536, 2048), np.arange(1024, 1536),
])


def _bn_fold(g, b, m, v):
    sc = g / np.sqrt(v + EPS)
    sh = b - m * sc
    return sc.astype(np.float32), sh.astype(np.float32)


def _bf16(a):
    import ml_dtypes
    return np.ascontiguousarray(np.asarray(a, np.float32).astype(
        ml_dtypes.bfloat16))


def prep_weights(inp):
    """Host-side constant folding. Returns dict of prepared arrays."""
    f = np.float32
    o = {}
    o['W0p'] = np.ascontiguousarray(inp['W0'][:, _PERM], f)
    o['b0p'] = np.ascontiguousarray(inp['b0'][_PERM], f)
    o['U0b'] = _bf16(inp['U0'][:, _PERM])
    o['U1b'] = _bf16(inp['U1'][:, _PERM])
    o['U2b'] = _bf16(inp['U2'][:, _PERM])
    sc0, sh0 = _bn_fold(inp['bn0_g'], inp['bn0_b'], inp['bn0_m'], inp['bn0_v'])
    o['W1b'] = _bf16((sc0[:, None] * inp['W1'])[:, _PERM])
    o['b1p'] = np.ascontiguousarray((inp['b1'] + sh0 @ inp['W1'])[_PERM], f)
    sc1, sh1 = _bn_fold(inp['bn1_g'], inp['bn1_b'], inp['bn1_m'], inp['bn1_v'])
    o['W2b'] = _bf16((sc1[:, None] * inp['W2'])[:, _PERM])
    o['b2p'] = np.ascontiguousarray((inp['b2'] + sh1 @ inp['W2'])[_PERM], f)
    o['Wab'] = _bf16(inp['Wa'])
    o['ba'] = np.ascontiguousarray(inp['ba'], f)
    # pooled = sum_t a*h2 (no 1/T); fold 1/T into Wd1
    o['Wd1p'] = np.ascontiguousarray(inp['Wd1'] / np.float32(T), f)
    o['bd1'] = np.ascontiguousarray(inp['bd1'], f)
    sc2, sh2 = _bn_fold(inp['bn2_g'], inp['bn2_b'], inp['bn2_m'], inp['bn2_v'])
    o['Wd2p'] = np.ascontiguousarray(sc2[:, None] * inp['Wd2'], f)
    o['bd2p'] = np.ascontiguousarray(inp['bd2'] + sh2 @ inp['Wd2'], f)
    o['Wd3'] = np.ascontiguousarray(inp['Wd3'], f)
    o['bd3'] = np.ascontiguousarray(inp['bd3'], f)
    # selector for summing rows (t,b) -> b : sel[p, b] = 1 if p % BL == b
    sel = np.zeros((P, BL), f)
    sel[np.arange(P), np.arange(P) % BL] = 1.0
    o['sel'] = sel
    o['ident'] = np.eye(P, dtype=f)
    return o


def _sigmoid(x):
    return 1.0 / (1.0 + np.exp(-x))


def numpy_forward(inp, t_steps=T, b_rows=B):
    """Numpy mirror of the kernel math (folded weights, permuted gates),
    in fp32 (no bf16 effects). Validates the host-side folds."""
    w = prep_weights(inp)
    x = np.asarray(inp['x'], np.float32)[:b_rows, :t_steps]
    U = {0: np.asarray(w['U0b'], np.float32),
         1: np.asarray(w['U1b'], np.float32),
         2: np.asarray(w['U2b'], np.float32)}
    W1 = np.asarray(w['W1b'], np.float32)
    W2 = np.asarray(w['W2b'], np.float32)

    def scan(xz, Um):
        bsz = xz.shape[0]
        h = np.zeros((bsz, H), np.float32)
        c = np.zeros((bsz, H), np.float32)
        hs = np.empty((bsz, t_steps, H), np.float32)
        for t in range(t_steps):
            z = xz[:, t] + h @ Um
            i = _sigmoid(z[:, 0:512]); f = _sigmoid(z[:, 512:1024])
            o_ = _sigmoid(z[:, 1024:1536]); g = np.tanh(z[:, 1536:2048])
            c = f * c + i * g
            h = o_ * np.tanh(c)
            hs[:, t] = h
        return hs  # [B, T, H]

    xz0 = np.einsum('btd,dg->btg', x, w['W0p']) + w['b0p']
    h0 = scan(xz0, U[0])
    xz1 = np.einsum('bth,hg->btg', h0, W1) + w['b1p']
    h1 = scan(xz1, U[1])
    xz2 = np.einsum('bth,hg->btg', h1, W2) + w['b2p']
    h2 = scan(xz2, U[2])

    e = np.tanh(np.einsum('bth,hk->btk', h2, np.asarray(w['Wab'], np.float32))
                + w['ba'])
    s = e.sum(-1)
    s = s - s.max(axis=1, keepdims=True)
    a = np.exp(s); a = a / a.sum(axis=1, keepdims=True)
    pooled = np.einsum('bt,bth->bh', a, h2)
    d1 = np.maximum(pooled @ w['Wd1p'] + w['bd1'], 0)
    d2 = np.maximum(d1 @ w['Wd2p'] + w['bd2p'], 0)
    return d2 @ w['Wd3'] + w['bd3']


# ---------------------------------------------------------------------------
# Bass program
# ---------------------------------------------------------------------------

def build_nc(t_steps=T):
    import concourse.bacc as bacc
    import concourse.mybir as mybir
    import concourse.tile as tile
    from contextlib import ExitStack

    f32 = mybir.dt.float32
    f32r = mybir.dt.float32r
    bf16 = mybir.dt.bfloat16
    AF = mybir.ActivationFunctionType
    OP = mybir.AluOpType
    M = t_steps * BL
    MT = M // P
    TPB = P // BL  # timesteps per 128-row tile (8)
    NSLOT = t_steps + LAG2

    nc = bacc.Bacc("TRN2", target_bir_lowering=False, debug=False,
                   num_devices=NC)

    def din(name, shape, dt=f32):
        return nc.dram_tensor(name, list(shape), dt, kind="ExternalInput")

    x_d = din('xT', (D, t_steps, BL))
    W0p = din('W0p', (D, G4)); b0p = din('b0p', (G4,))
    U_d = [din('U0b', (H, G4), bf16), din('U1b', (H, G4), bf16),
           din('U2b', (H, G4), bf16)]
    W_d = {1: din('W1b', (H, G4), bf16), 2: din('W2b', (H, G4), bf16)}
    bias_d = {1: din('b1p', (G4,)), 2: din('b2p', (G4,))}
    Wab = din('Wab', (H, H), bf16); ba = din('ba', (H,))
    Wd1p = din('Wd1p', (H, P)); bd1 = din('bd1', (P,))
    Wd2p = din('Wd2p', (P, 64)); bd2p = din('bd2p', (64,))
    Wd3 = din('Wd3', (64, 5)); bd3 = din('bd3', (5,))
    sel_d = din('sel', (P, BL))
    ident_d = din('ident', (P, P))
    outT = nc.dram_tensor('outT', [5, BL], f32, kind="ExternalOutput")

    # DRAM temps
    xz_d = nc.dram_tensor('xz_buf', [M, G4], bf16)
    h2T = nc.dram_tensor('h2T', [4, P, t_steps, BL], bf16)
    h2rows = nc.dram_tensor('h2rows', [M, H], bf16)
    s_dram = nc.dram_tensor('s_dram', [M], f32)
    a_dram = nc.dram_tensor('a_dram', [M], f32)

    NSL = [slice(n * 512, (n + 1) * 512) for n in range(4)]
    ROWS = [slice(32 * l, 32 * l + BL) for l in range(3)]

    with tile.TileContext(nc) as tc:
        with ExitStack() as gctx:
            gconst = gctx.enter_context(tc.tile_pool(name="gconst", bufs=1))
            ident = gconst.tile([P, P], f32)
            nc.sync.dma_start(ident[:], ident_d[:, :])
            sel = gconst.tile([P, BL], f32)
            nc.sync.dma_start(sel[:], sel_d[:, :])

            # ---------------- layer-0 input projection pass ----------------
            def xz_pass():
                with ExitStack() as ctx:
                    cst = ctx.enter_context(tc.tile_pool(name="p0c", bufs=1))
                    W_stg = cst.tile([P, G4], f32, name="p0Ws")
                    nc.sync.dma_start(W_stg[:], W0p[:, :])
                    W_sb = cst.tile([P, G4], f32r, name="p0W")
                    nc.any.tensor_copy(W_sb[:], W_stg[:])
                    brep = cst.tile([P, G4], f32, name="p0b")
                    nc.sync.dma_start(
                        brep[:], b0p[None, :].to_broadcast((P, G4)))
                    io = ctx.enter_context(tc.tile_pool(name="p0io", bufs=3))
                    ps = ctx.enter_context(
                        tc.tile_pool(name="p0ps", bufs=2, space="PSUM"))
                    for m in range(MT):
                        km_s = io.tile([P, P], f32, tag="km_s")
                        nc.sync.dma_start(
                            km_s[:].rearrange("p (t b) -> p t b", b=BL),
                            x_d[:, m * TPB:(m + 1) * TPB, :])
                        km = io.tile([P, P], f32r, tag="km")
                        nc.any.tensor_copy(km[:], km_s[:])
                        zp = ps.tile([P, G4], f32, tag="zp")
                        for n in range(4):
                            nc.tensor.matmul(zp[:, NSL[n]], km[:],
                                             W_sb[:, NSL[n]],
                                             start=True, stop=True)
                        ob = io.tile([P, G4], bf16, tag="ob")
                        nc.vector.tensor_tensor(ob[:], zp[:], brep[:], OP.add)
                        nc.sync.dma_start(xz_d[m * P:(m + 1) * P, :], ob[:])

            # ---------------- 3-layer wavefront scan ----------------
            def wavefront():
                with ExitStack() as ctx:
                    cst = ctx.enter_context(tc.tile_pool(name="wfc", bufs=1))
                    U_sb = []
                    for l in range(3):
                        u = cst.tile([P, 4, G4], bf16, name=f"wfU{l}")
                        nc.sync.dma_start(
                            u[:], U_d[l].rearrange("(k p) n -> p k n", p=P))
                        U_sb.append(u)
                    W_sb = {}
                    for l in (1, 2):
                        w = cst.tile([P, 4, G4], bf16, name=f"wfW{l}")
                        nc.sync.dma_start(
                            w[:], W_d[l].rearrange("(k p) n -> p k n", p=P))
                        W_sb[l] = w
                    brep = {}
                    for l in (1, 2):
                        bt_s = cst.tile([P, G4], f32, name=f"wfbrs{l}")
                        nc.sync.dma_start(
                            bt_s[:], bias_d[l][None, :].to_broadcast((P, G4)))
                        bt = cst.tile([P, G4], bf16, name=f"wfbr{l}")
                        nc.any.tensor_copy(bt[:], bt_s[:])
                        brep[l] = bt
                    # persistent state
                    c_sb = cst.tile([80, H], f32, name="wf_c")
                    nc.vector.memset(c_sb[:], 0.0)
                    h_bf = cst.tile([96, H], bf16, name="wf_h")
                    nc.vector.memset(h_bf[:], 0.0)
                    tmp_bf = cst.tile([96, H], bf16, name="wf_tmp")
                    nc.vector.memset(tmp_bf[:], 0.0)
                    hT = [cst.tile([P, 4, HR, BL], bf16, name=f"wfhT{l}")
                          for l in range(3)]
                    for l in range(3):
                        nc.vector.memset(hT[l][:], 0.0)
                    ring = [cst.tile([80, G4], bf16, name=f"wfring{r}")
                            for r in range(RING)]
                    for r in range(RING):
                        nc.vector.memset(ring[r][:], 0.0)
                    stage = cst.tile([P, G4], bf16, name="wf_stage")
                    psp = ctx.enter_context(
                        tc.tile_pool(name="wfps", bufs=1, space="PSUM"))
                    z_ps = psp.tile([P, G4], f32, name="wf_zps")
                    nc.vector.memset(z_ps[:], 0.0)
                    pj_ps = psp.tile([P, G4], f32, name="wf_pjps")
                    nc.vector.memset(pj_ps[:], 0.0)
                    wk = ctx.enter_context(tc.tile_pool(name="wfwk", bufs=2))

                    def prefetch_xz0(u):
                        if 0 <= u < t_steps:
                            nc.gpsimd.dma_start(
                                ring[u % RING][0:BL, :],
                                xz_d[u * BL:(u + 1) * BL, :])

                    for u in range(4):
                        prefetch_xz0(u)

                    for s in range(NSLOT):
                        ts_ = [s, s - LAG1, s - LAG2]
                        act = [0 <= t < t_steps for t in ts_]
                        rec = [act[l] and ts_[l] >= 1 for l in range(3)]
                        steady = all(act)
                        prefetch_xz0(s + 4)

                        # projection schedule: two steps of one layer/slot
                        pj = None
                        if s % 2 == 0:
                            tau = s - 4
                            if 0 <= tau and tau + 1 < t_steps:
                                pj = (1, tau)
                        else:
                            tau = s - 13
                            if 0 <= tau and tau + 1 < t_steps:
                                pj = (2, tau)

                        # ---- matmuls, 4 col groups interleaved ----
                        for n in range(4):
                            for k in range(4):
                                for l in range(3):
                                    if rec[l]:
                                        nc.tensor.matmul(
                                            z_ps[ROWS[l], NSL[n]],
                                            hT[l][:, k, (ts_[l] - 1) % HR, :],
                                            U_sb[l][:, k, NSL[n]],
                                            start=(k == 0), stop=(k == 3),
                                            tile_position=(0, 32 * l))
                                if pj is not None:
                                    L, tau = pj
                                    ti = tau % HR
                                    nc.tensor.matmul(
                                        pj_ps[96:128, NSL[n]],
                                        hT[L - 1][:, k, ti:ti + 2, :],
                                        W_sb[L][:, k, NSL[n]],
                                        start=(k == 0), stop=(k == 3),
                                        tile_position=(0, 96))
                            # ---- z = psum + xz for chunk n ----
                            if n == 0:
                                z_sb = wk.tile([80, G4], f32, tag="zsb")
                            rg = ring[s % RING]
                            if steady and all(rec):
                                nc.vector.tensor_tensor(
                                    z_sb[0:80, NSL[n]], z_ps[0:80, NSL[n]],
                                    rg[0:80, NSL[n]], OP.add)
                            else:
                                for l in range(3):
                                    if not act[l]:
                                        continue
                                    if rec[l]:
                                        nc.vector.tensor_tensor(
                                            z_sb[ROWS[l], NSL[n]],
                                            z_ps[ROWS[l], NSL[n]],
                                            rg[ROWS[l], NSL[n]], OP.add)
                                    else:
                                        nc.vector.tensor_copy(
                                            z_sb[ROWS[l], NSL[n]],
                                            rg[ROWS[l], NSL[n]])

                        # ---- gates ----
                        sig = wk.tile([80, 3 * H], f32, tag="sig")
                        g_t = wk.tile([80, H], f32, tag="g")
                        ig = wk.tile([80, H], f32, tag="ig")
                        tch = wk.tile([80, H], f32, tag="tch")
                        if steady and all(rec):
                            nc.scalar.activation(sig[:], z_sb[0:80, 0:3 * H],
                                                 AF.Sigmoid)
                            nc.scalar.activation(g_t[:], z_sb[0:80, 3 * H:G4],
                                                 AF.Tanh)
                            nc.vector.tensor_tensor(
                                c_sb[:], c_sb[:], sig[:, H:2 * H], OP.mult)
                            nc.vector.tensor_tensor(
                                ig[:], sig[:, 0:H], g_t[:], OP.mult)
                            nc.vector.tensor_tensor(
                                c_sb[:], c_sb[:], ig[:], OP.add)
                            nc.scalar.activation(tch[:], c_sb[:], AF.Tanh)
                            nc.vector.tensor_tensor(
                                h_bf[0:80, :], sig[:, 2 * H:3 * H], tch[:],
                                OP.mult)
                        else:
                            for l in range(3):
                                if not act[l]:
                                    continue
                                r = ROWS[l]
                                nc.scalar.activation(
                                    sig[r, :], z_sb[r, 0:3 * H], AF.Sigmoid)
                                nc.scalar.activation(
                                    g_t[r, :], z_sb[r, 3 * H:G4], AF.Tanh)
                                if ts_[l] == 0:
                                    nc.vector.tensor_tensor(
                                        c_sb[r, :], sig[r, 0:H], g_t[r, :],
                                        OP.mult)
                                else:
                                    nc.vector.tensor_tensor(
                                        c_sb[r, :], c_sb[r, :],
                                        sig[r, H:2 * H], OP.mult)
                                    nc.vector.tensor_tensor(
                                        ig[r, :], sig[r, 0:H], g_t[r, :],
                                        OP.mult)
                                    nc.vector.tensor_tensor(
                                        c_sb[r, :], c_sb[r, :], ig[r, :],
                                        OP.add)
                                nc.scalar.activation(
                                    tch[r, :], c_sb[r, :], AF.Tanh)
                                nc.vector.tensor_tensor(
                                    h_bf[r, :], sig[r, 2 * H:3 * H],
                                    tch[r, :], OP.mult)

                        # ---- transpose h and scatter into hT rings ----
                        nc.vector.transpose(tmp_bf[:], h_bf[:])
                        seq = [nc.gpsimd, nc.scalar, nc.sync]
                        for l in range(3):
                            if not act[l]:
                                continue
                            for j in range(4):
                                # in: cols 32j+128kk+b over kk,b; out: rows
                                # 32j..32j+32, all kk, one time slot
                                seq[l].dma_start(
                                    hT[l][32 * j:32 * j + 32, :,
                                          ts_[l] % HR, :],
                                    tmp_bf[32 * l:32 * l + 32, :]
                                    .rearrange("p (k c) -> p k c", c=P)
                                    [:, :, 32 * j:32 * j + BL])

                        # ---- layer-2 outputs for attention ----
                        if act[2]:
                            t2 = ts_[2]
                            nc.sync.dma_start(
                                h2rows[t2 * BL:(t2 + 1) * BL, :],
                                h_bf[64:64 + BL, :])
                            nc.sync.dma_start(
                                h2T.rearrange("k p t b -> p k t b")[:, :, t2, :],
                                hT[2][:, :, t2 % HR, :])

                        # ---- evict projection (bias added here) ----
                        if pj is not None:
                            L, tau = pj
                            lag = LAG1 if L == 1 else LAG2
                            nc.vector.tensor_tensor(
                                stage[96:128, :], pj_ps[96:128, :],
                                brep[L][96:128, :], OP.add)
                            for i in range(2):
                                dst = ring[(tau + i + lag) % RING]
                                nc.scalar.dma_start(
                                    dst[ROWS[L], :],
                                    stage[96 + BL * i:96 + BL * (i + 1), :])

            # ---------------- run pipeline ----------------
            xz_pass()
            wavefront()

            # ---------------- attention ----------------
            with ExitStack() as ctx:
                cst = ctx.enter_context(tc.tile_pool(name="atc", bufs=1))
                Wa_sb = cst.tile([P, 4, H], bf16, name="atWa")
                nc.sync.dma_start(
                    Wa_sb[:], Wab.rearrange("(k p) n -> p k n", p=P))
                ba_rep = cst.tile([P, H], f32)
                nc.sync.dma_start(ba_rep[:], ba[None, :].to_broadcast((P, H)))
                s_sb = cst.tile([P, MT], f32)
                io = ctx.enter_context(tc.tile_pool(name="atio", bufs=3))
                ps = ctx.enter_context(
                    tc.tile_pool(name="atps", bufs=2, space="PSUM"))
                # e-pass: s[(t,b)] = sum_k tanh(h2 @ Wa + ba)
                for m in range(MT):
                    kxm = io.tile([P, 4, TPB, BL], bf16, tag="kxm")
                    for k in range(4):
                        nc.sync.dma_start(
                            kxm[:, k],
                            h2T[k, :, m * TPB:(m + 1) * TPB, :])
                    ep = ps.tile([P, H], f32, tag="ep")
                    for k in range(4):
                        nc.tensor.matmul(
                            ep[:], kxm[:, k], Wa_sb[:, k, :],
                            start=(k == 0), stop=(k == 3))
                    e_sb = io.tile([P, H], f32, tag="e")
                    nc.vector.tensor_tensor(e_sb[:], ep[:], ba_rep[:], OP.add)
                    e_t = io.tile([P, H], f32, tag="et")
                    nc.scalar.activation(e_t[:], e_sb[:], AF.Tanh,
                                         accum_out=s_sb[:, m:m + 1])

                # s (row layout [P, MT]) -> sT [BL, t_steps] via flat DRAM
                nc.sync.dma_start(
                    s_dram.rearrange("(m p) -> p m", p=P), s_sb[:])
                sT = cst.tile([BL, t_steps], f32)
                nc.sync.dma_start(
                    sT[:], s_dram.rearrange("(t b) -> b t", b=BL))
                mx = cst.tile([BL, 1], f32)
                nc.vector.reduce_max(mx[:], sT[:], axis=mybir.AxisListType.X)
                nmx = cst.tile([BL, 1], f32)
                nc.vector.tensor_scalar_mul(nmx[:], mx[:], -1.0)
                ex = cst.tile([BL, t_steps], f32)
                sm = cst.tile([BL, 1], f32)
                nc.scalar.activation(ex[:], sT[:], AF.Exp, bias=nmx[:],
                                     accum_out=sm[:])
                rs = cst.tile([BL, 1], f32)
                nc.vector.reciprocal(rs[:], sm[:])
                aT = cst.tile([BL, t_steps], f32)
                nc.vector.tensor_scalar_mul(aT[:], ex[:], rs[:])
                nc.sync.dma_start(
                    a_dram.rearrange("(t b) -> b t", b=BL), aT[:])
                a_row = cst.tile([P, MT], f32)
                nc.sync.dma_start(
                    a_row[:], a_dram.rearrange("(m p) -> p m", p=P))

                # pooled[b, :] = sum_rows sel * (a * h2)   (f32 matmuls)
                pp = ctx.enter_context(
                    tc.tile_pool(name="atpp", bufs=1, space="PSUM"))
                ps1 = ctx.enter_context(
                    tc.tile_pool(name="atp1", bufs=1, space="PSUM"))
                pooled_ps = pp.tile([BL, H], f32)
                for m in range(MT):
                    h2t = io.tile([P, H], bf16, tag="h2t")
                    nc.sync.dma_start(h2t[:], h2rows[m * P:(m + 1) * P, :])
                    wrow = io.tile([P, H], f32, tag="wrow")
                    nc.vector.tensor_scalar_mul(wrow[:], h2t[:],
                                                a_row[:, m:m + 1])
                    nc.tensor.matmul(pooled_ps[:], sel[:], wrow[:],
                                     start=(m == 0), stop=(m == MT - 1))

                # pooledT via PE transpose
                pooled_sb = cst.tile([BL, H], f32)
                nc.vector.tensor_copy(pooled_sb[:], pooled_ps[:])
                ptp = ps1.tile([P, 4 * BL], f32, tag="ptp")
                for k in range(4):
                    nc.tensor.transpose(
                        ptp[:, k * BL:(k + 1) * BL],
                        pooled_sb[:, k * P:(k + 1) * P], ident[0:BL, 0:BL])
                pooledT = cst.tile([P, 4, BL], f32r)
                nc.vector.tensor_copy(
                    pooledT[:], ptp[:].rearrange("p (k b) -> p k b", k=4))

                # ---------------- dense head ----------------
                def load_r(pool, dram_ap, shape, name):
                    stg = pool.tile(shape, f32, name=name + "_stg")
                    nc.sync.dma_start(stg[:], dram_ap)
                    t_ = pool.tile(shape, f32r, name=name)
                    nc.any.tensor_copy(t_[:], stg[:])
                    return t_

                Wd1_sb = load_r(cst, Wd1p.rearrange("(k p) n -> p k n", p=P),
                                [P, 4, P], "hWd1")
                bd1_sb = cst.tile([P, 1], f32)
                nc.sync.dma_start(bd1_sb[:], bd1[:, None])
                Wd2_sb = load_r(cst, Wd2p[:, :], [P, 64], "hWd2")
                bd2_sb = cst.tile([64, 1], f32)
                nc.sync.dma_start(bd2_sb[:], bd2p[:, None])
                Wd3_sb = load_r(cst, Wd3[:, :], [64, 5], "hWd3")
                bd3_sb = cst.tile([5, 1], f32)
                nc.sync.dma_start(bd3_sb[:], bd3[:, None])

                d1p = ps1.tile([P, BL], f32, tag="d1p")
                for k in range(4):
                    nc.tensor.matmul(d1p[:], Wd1_sb[:, k, :], pooledT[:, k, :],
                                     start=(k == 0), stop=(k == 3))
                d1 = cst.tile([P, BL], f32r)
                nc.scalar.activation(d1[:], d1p[:], AF.Relu, bias=bd1_sb[:])
                d2p = ps1.tile([64, BL], f32, tag="d2p")
                nc.tensor.matmul(d2p[:], Wd2_sb[:], d1[:], start=True,
                                 stop=True)
                d2 = cst.tile([64, BL], f32r)
                nc.scalar.activation(d2[:], d2p[:], AF.Relu, bias=bd2_sb[:])
                d3p = ps1.tile([5, BL], f32, tag="d3p")
                nc.tensor.matmul(d3p[:], Wd3_sb[:], d2[:], start=True,
                                 stop=True)
                d3 = cst.tile([5, BL], f32)
                nc.scalar.activation(d3[:], d3p[:], AF.Identity, bias=bd3_sb[:])
                nc.sync.dma_start(outT[:, :], d3[:])

    nc.compile()
    return nc


@functools.lru_cache(maxsize=2)
def _compiled(t_steps):
    return build_nc(t_steps)


def _make_in_maps(inputs):
    w = prep_weights(inputs)
    x = np.ascontiguousarray(np.asarray(inputs['x'], np.float32))
    base = {k: w[k] for k in (
        'W0p', 'b0p', 'U0b', 'U1b', 'U2b', 'W1b', 'b1p', 'W2b', 'b2p',
        'Wab', 'ba', 'Wd1p', 'bd1', 'Wd2p', 'bd2p', 'Wd3', 'bd3', 'sel',
        'ident')}
    in_maps = []
    for c in range(NC):
        m = dict(base)
        m['xT'] = np.ascontiguousarray(
            x[c * BL:(c + 1) * BL].transpose(2, 1, 0))
        in_maps.append(m)
    return in_maps


def kernel(**inputs):
    from concourse import bass_utils
    nc = _compiled(T)
    in_maps = _make_in_maps(inputs)
    res = bass_utils.run_bass_kernel_spmd(nc, in_maps, core_ids=list(range(NC)))
    out = np.concatenate([np.asarray(res.results[c]['outT']).T
                          for c in range(NC)], axis=0)
    return np.ascontiguousarray(out, np.float32)


def timed_run(tmpdir=None, **inputs):
    """Run with NTFF profiling; returns BassKernelResults."""
    from concourse import bass_utils
    nc = _compiled(T)
    in_maps = _make_in_maps(inputs)
    res = bass_utils.run_bass_kernel_spmd(
        nc, in_maps, core_ids=list(range(NC)), trace=True, tmpdir=tmpdir)
    return res



# revision 13
# speedup vs baseline: 1.4197x; 1.4197x over previous
"""Trainium2 Bass kernel for stacked-LSTM + attention + dense head.

Model (per reference):
  3x LSTM(H=512, return_sequences) with inference BatchNorm between layers,
  attention pooling over time, then Dense(128)+BN+Dense(64)+Dense(5).
  B=128, T=512, D=128, H=512, fp32.

Strategy: data-parallel over batch (16 rows/core on 8 cores). Per core a
3-layer *wavefront*: slot s advances layer0 at t=s, layer1 at t=s-10,
layer2 at t=s-20. Gate banks are ordered [f|g|i|o] (one PSUM bank per
gate) so activations read each bank directly from PSUM as soon as its
matmuls retire. xz input projections are injected into PSUM by the PE
itself (identity / row-selector matmuls), eliminating the DVE adds.
h is re-transposed each step with 4 PE-transposes + one DVE copy into
an SBUF ring (hTr) holding h^T for all 3 layers with time as a free
dim; recurrent and projection matmuls read it directly as stationary
operands. The three recurrent lanes + the projection lane run in
separate PE column groups (tile_position) concurrently.

Self-contained: hardcodes shapes; no reads of reference.py/spec.json.
"""

import functools

import numpy as np

B, T, D, H = 128, 512, 128, 512
NC = 8
BL = B // NC          # batch rows per core
G4 = 4 * H            # gate width 2048
EPS = 1e-3
P = 128
LAG1, LAG2 = 10, 20   # wavefront lags of layers 1 and 2
RING = 8              # xz0 ring slots
HR = 8                # hTr time-ring length (slot-indexed)
NPJR = 6              # staged-projection ring pairs per layer

# column permutation: keras gate order [i|f|c|o] -> kernel order [f|g|i|o]
_PERM = np.concatenate([
    np.arange(512, 1024),    # f
    np.arange(1024, 1536),   # g (keras 'c' gate)
    np.arange(0, 512),       # i
    np.arange(1536, 2048),   # o
])


def _bn_fold(g, b, m, v):
    sc = g / np.sqrt(v + EPS)
    sh = b - m * sc
    return sc.astype(np.float32), sh.astype(np.float32)


def _bf16(a):
    import ml_dtypes
    return np.ascontiguousarray(np.asarray(a, np.float32).astype(
        ml_dtypes.bfloat16))


def prep_weights(inp):
    """Host-side constant folding. Returns dict of prepared arrays."""
    f = np.float32
    o = {}
    o['W0p'] = np.ascontiguousarray(inp['W0'][:, _PERM], f)
    o['b0p'] = np.ascontiguousarray(inp['b0'][_PERM], f)
    o['U0b'] = _bf16(inp['U0'][:, _PERM])
    o['U1b'] = _bf16(inp['U1'][:, _PERM])
    o['U2b'] = _bf16(inp['U2'][:, _PERM])
    sc0, sh0 = _bn_fold(inp['bn0_g'], inp['bn0_b'], inp['bn0_m'], inp['bn0_v'])
    o['W1b'] = _bf16((sc0[:, None] * inp['W1'])[:, _PERM])
    o['b1pb'] = _bf16((inp['b1'] + sh0 @ inp['W1'])[_PERM])[None, :]
    sc1, sh1 = _bn_fold(inp['bn1_g'], inp['bn1_b'], inp['bn1_m'], inp['bn1_v'])
    o['W2b'] = _bf16((sc1[:, None] * inp['W2'])[:, _PERM])
    o['b2pb'] = _bf16((inp['b2'] + sh1 @ inp['W2'])[_PERM])[None, :]
    o['Wab'] = _bf16(inp['Wa'])
    o['ba'] = np.ascontiguousarray(inp['ba'], f)
    # pooled = sum_t a*h2 (no 1/T); fold 1/T into Wd1
    o['Wd1p'] = np.ascontiguousarray(inp['Wd1'] / np.float32(T), f)
    o['bd1'] = np.ascontiguousarray(inp['bd1'], f)
    sc2, sh2 = _bn_fold(inp['bn2_g'], inp['bn2_b'], inp['bn2_m'], inp['bn2_v'])
    o['Wd2p'] = np.ascontiguousarray(sc2[:, None] * inp['Wd2'], f)
    o['bd2p'] = np.ascontiguousarray(inp['bd2'] + sh2 @ inp['Wd2'], f)
    o['Wd3'] = np.ascontiguousarray(inp['Wd3'], f)
    o['bd3'] = np.ascontiguousarray(inp['bd3'], f)
    # selector for summing rows (t,b) -> b : sel[p, b] = 1 if p % BL == b
    sel = np.zeros((P, BL), f)
    sel[np.arange(P), np.arange(P) % BL] = 1.0
    o['sel'] = sel
    o['ident'] = np.eye(P, dtype=f)
    # small bf16 constants for PE-side injection / transpose
    o['id16b'] = _bf16(np.eye(16))
    ev = np.zeros((32, 16), f); ev[np.arange(16), np.arange(16)] = 1.0
    od = np.zeros((32, 16), f); od[16 + np.arange(16), np.arange(16)] = 1.0
    o['sel_ev'] = _bf16(ev)
    o['sel_od'] = _bf16(od)
    o['ones1'] = _bf16(np.ones((1, 32)))
    o['id96b'] = _bf16(np.eye(96))
    return o


def _sigmoid(x):
    return 1.0 / (1.0 + np.exp(-x))


def numpy_forward(inp, t_steps=T, b_rows=B):
    """Numpy mirror of the kernel math (folded weights, permuted gates),
    in fp32 (no bf16 effects). Validates the host-side folds."""
    w = prep_weights(inp)
    x = np.asarray(inp['x'], np.float32)[:b_rows, :t_steps]
    U = {0: np.asarray(w['U0b'], np.float32),
         1: np.asarray(w['U1b'], np.float32),
         2: np.asarray(w['U2b'], np.float32)}
    W1 = np.asarray(w['W1b'], np.float32)
    W2 = np.asarray(w['W2b'], np.float32)

    def scan(xz, Um):
        bsz = xz.shape[0]
        h = np.zeros((bsz, H), np.float32)
        c = np.zeros((bsz, H), np.float32)
        hs = np.empty((bsz, t_steps, H), np.float32)
        for t in range(t_steps):
            z = xz[:, t] + h @ Um
            f = _sigmoid(z[:, 0:512]); g = np.tanh(z[:, 512:1024])
            i = _sigmoid(z[:, 1024:1536]); o_ = _sigmoid(z[:, 1536:2048])
            c = f * c + i * g
            h = o_ * np.tanh(c)
            hs[:, t] = h
        return hs  # [B, T, H]

    xz0 = np.einsum('btd,dg->btg', x, w['W0p']) + w['b0p']
    h0 = scan(xz0, U[0])
    xz1 = np.einsum('bth,hg->btg', h0, W1) + np.asarray(w['b1pb'], np.float32)
    h1 = scan(xz1, U[1])
    xz2 = np.einsum('bth,hg->btg', h1, W2) + np.asarray(w['b2pb'], np.float32)
    h2 = scan(xz2, U[2])

    e = np.tanh(np.einsum('bth,hk->btk', h2, np.asarray(w['Wab'], np.float32))
                + w['ba'])
    s = e.sum(-1)
    s = s - s.max(axis=1, keepdims=True)
    a = np.exp(s); a = a / a.sum(axis=1, keepdims=True)
    pooled = np.einsum('bt,bth->bh', a, h2)
    d1 = np.maximum(pooled @ w['Wd1p'] + w['bd1'], 0)
    d2 = np.maximum(d1 @ w['Wd2p'] + w['bd2p'], 0)
    return d2 @ w['Wd3'] + w['bd3']


# ---------------------------------------------------------------------------
# Bass program
# ---------------------------------------------------------------------------

def build_nc(t_steps=T, debug=False):
    import concourse.bacc as bacc
    import concourse.mybir as mybir
    import concourse.tile as tile
    from contextlib import ExitStack

    f32 = mybir.dt.float32
    f32r = mybir.dt.float32r
    bf16 = mybir.dt.bfloat16
    AF = mybir.ActivationFunctionType
    OP = mybir.AluOpType
    M = t_steps * BL
    MT = M // P
    TPB = P // BL  # timesteps per 128-row tile (8)
    NSLOT = t_steps + LAG2

    nc = bacc.Bacc("TRN2", target_bir_lowering=False, debug=False,
                   num_devices=NC)

    def din(name, shape, dt=f32):
        return nc.dram_tensor(name, list(shape), dt, kind="ExternalInput")

    x_d = din('xT', (D, t_steps, BL))
    W0p = din('W0p', (D, G4)); b0p = din('b0p', (G4,))
    U_d = [din('U0b', (H, G4), bf16), din('U1b', (H, G4), bf16),
           din('U2b', (H, G4), bf16)]
    W_d = {1: din('W1b', (H, G4), bf16), 2: din('W2b', (H, G4), bf16)}
    bias_d = {1: din('b1pb', (1, G4), bf16), 2: din('b2pb', (1, G4), bf16)}
    Wab = din('Wab', (H, H), bf16); ba = din('ba', (H,))
    Wd1p = din('Wd1p', (H, P)); bd1 = din('bd1', (P,))
    Wd2p = din('Wd2p', (P, 64)); bd2p = din('bd2p', (64,))
    Wd3 = din('Wd3', (64, 5)); bd3 = din('bd3', (5,))
    sel_d = din('sel', (P, BL))
    ident_d = din('ident', (P, P))
    id16_d = din('id16b', (16, 16), bf16)
    selev_d = din('sel_ev', (32, 16), bf16)
    selod_d = din('sel_od', (32, 16), bf16)
    ones1_d = din('ones1', (1, 32), bf16)
    id96_d = din('id96b', (96, 96), bf16)
    outT = nc.dram_tensor('outT', [5, BL], f32, kind="ExternalOutput")

    # DRAM temps
    xz_d = nc.dram_tensor('xz_buf', [M, G4], bf16)
    h2T = nc.dram_tensor('h2T', [4, P, t_steps, BL], bf16,
                         kind="ExternalOutput" if debug else "Internal")
    h2rows = nc.dram_tensor('h2rows', [M, H], bf16,
                            kind="ExternalOutput" if debug else "Internal")
    if debug is True:
        h0rows = nc.dram_tensor('h0rows', [M, H], bf16,
                                kind="ExternalOutput")
        h1rows = nc.dram_tensor('h1rows', [M, H], bf16,
                                kind="ExternalOutput")
        xz1rows = nc.dram_tensor('xz1rows', [M, G4], bf16,
                                 kind="ExternalOutput")
    s_dram = nc.dram_tensor('s_dram', [M], f32,
                            kind="ExternalOutput" if debug else "Internal")
    a_dram = nc.dram_tensor('a_dram', [M], f32,
                            kind="ExternalOutput" if debug else "Internal")

    NSL = [slice(n * 512, (n + 1) * 512) for n in range(4)]
    ROWS = [slice(32 * l, 32 * l + BL) for l in range(3)]
    # gate bank order: 0=f, 1=g, 2=i, 3=o
    F_, G_, I_, O_ = NSL[0], NSL[1], NSL[2], NSL[3]

    with tile.TileContext(nc) as tc:
        with ExitStack() as gctx:
            gconst = gctx.enter_context(tc.tile_pool(name="gconst", bufs=1))
            ident = gconst.tile([P, P], f32)
            nc.sync.dma_start(ident[:], ident_d[:, :])
            sel = gconst.tile([P, BL], f32)
            nc.sync.dma_start(sel[:], sel_d[:, :])

            # ---------------- layer-0 input projection pass ----------------
            def xz_pass():
                with ExitStack() as ctx:
                    cst = ctx.enter_context(tc.tile_pool(name="p0c", bufs=1))
                    W_stg = cst.tile([P, G4], f32, name="p0Ws")
                    nc.sync.dma_start(W_stg[:], W0p[:, :])
                    W_sb = cst.tile([P, G4], f32r, name="p0W")
                    nc.any.tensor_copy(W_sb[:], W_stg[:])
                    brep = cst.tile([P, G4], f32, name="p0b")
                    nc.sync.dma_start(
                        brep[:], b0p[None, :].to_broadcast((P, G4)))
                    io = ctx.enter_context(tc.tile_pool(name="p0io", bufs=3))
                    ps = ctx.enter_context(
                        tc.tile_pool(name="p0ps", bufs=2, space="PSUM"))
                    for m in range(MT):
                        km_s = io.tile([P, P], f32, tag="km_s")
                        nc.sync.dma_start(
                            km_s[:].rearrange("p (t b) -> p t b", b=BL),
                            x_d[:, m * TPB:(m + 1) * TPB, :])
                        km = io.tile([P, P], f32r, tag="km")
                        nc.any.tensor_copy(km[:], km_s[:])
                        zp = ps.tile([P, G4], f32, tag="zp")
                        for n in range(4):
                            nc.tensor.matmul(zp[:, NSL[n]], km[:],
                                             W_sb[:, NSL[n]],
                                             start=True, stop=True)
                        ob = io.tile([P, G4], bf16, tag="ob")
                        nc.vector.tensor_tensor(ob[:], zp[:], brep[:], OP.add)
                        nc.sync.dma_start(xz_d[m * P:(m + 1) * P, :], ob[:])

            # ---------------- 3-layer wavefront scan ----------------
            LAGS = [0, LAG1, LAG2]
            HA, HB = slice(0, 256), slice(256, 512)

            def wavefront():
                with ExitStack() as ctx:
                    cst = ctx.enter_context(tc.tile_pool(name="wfc", bufs=1))
                    U_sb = []
                    for l in range(3):
                        u = cst.tile([P, 4, G4], bf16, name=f"wfU{l}")
                        nc.sync.dma_start(
                            u[:], U_d[l].rearrange("(k p) n -> p k n", p=P))
                        U_sb.append(u)
                    W_sb = {}
                    for l in (1, 2):
                        w = cst.tile([P, 4, G4], bf16, name=f"wfW{l}")
                        nc.sync.dma_start(
                            w[:], W_d[l].rearrange("(k p) n -> p k n", p=P))
                        W_sb[l] = w
                    biasrow = {}
                    for l in (1, 2):
                        bt = cst.tile([1, G4], bf16, name=f"wfbr{l}")
                        nc.sync.dma_start(bt[:], bias_d[l][:, :])
                        biasrow[l] = bt
                    id16 = cst.tile([16, 16], bf16, name="wfid16")
                    nc.sync.dma_start(id16[:], id16_d[:, :])
                    sel_ev = cst.tile([32, 16], bf16, name="wfsev")
                    nc.sync.dma_start(sel_ev[:], selev_d[:, :])
                    sel_od = cst.tile([32, 16], bf16, name="wfsod")
                    nc.sync.dma_start(sel_od[:], selod_d[:, :])
                    ones1 = cst.tile([1, 32], bf16, name="wfones")
                    nc.sync.dma_start(ones1[:], ones1_d[:, :])
                    id96 = cst.tile([96, 96], bf16, name="wfid96")
                    nc.sync.dma_start(id96[:], id96_d[:, :])

                    # persistent state
                    c_sb = cst.tile([80, H], f32, name="wf_c")
                    nc.vector.memset(c_sb[:], 0.0)
                    h_bf = cst.tile([96, H], bf16, name="wf_h")
                    nc.vector.memset(h_bf[:], 0.0)
                    hTr = cst.tile([P, 4, 3, HR, BL], bf16, name="wf_hTr")
                    nc.vector.memset(hTr[:], 0.0)
                    ring0 = [cst.tile([BL, G4], bf16, name=f"wfring{r}")
                             for r in range(RING)]
                    for r in range(RING):
                        nc.vector.memset(ring0[r][:], 0.0)
                    pjring = {l: [cst.tile([32, G4], bf16, name=f"wfpj{l}_{r}")
                                  for r in range(NPJR)] for l in (1, 2)}
                    for l in (1, 2):
                        for r in range(NPJR):
                            nc.vector.memset(pjring[l][r][:], 0.0)

                    psp = ctx.enter_context(
                        tc.tile_pool(name="wfps", bufs=1, space="PSUM"))
                    z_ps = psp.tile([P, G4], f32, name="wf_zps")
                    nc.vector.memset(z_ps[:], 0.0)
                    pj_ps = psp.tile([P, 1024], f32, name="wf_pjps")
                    nc.vector.memset(pj_ps[:], 0.0)
                    tr_ps = psp.tile([P, 4, 96], bf16, name="wf_trps")
                    wk = ctx.enter_context(tc.tile_pool(name="wfwk", bufs=2))

                    def prefetch_xz0(u):
                        if 0 <= u < t_steps:
                            nc.gpsimd.dma_start(
                                ring0[u % RING][:, :],
                                xz_d[u * BL:(u + 1) * BL, :])

                    for u in range(4):
                        prefetch_xz0(u)

                    # pj PSUM placement per gate chunk: (row base, col
                    # slice) -> PE column lanes 96/64/0/32, 2 PSUM banks
                    PJP = [(96, slice(0, 512)), (64, slice(512, 1024)),
                           (0, slice(0, 512)), (32, slice(512, 1024))]

                    pending_evict = []

                    def flush_evicts():
                        while pending_evict:
                            dst, _pj = pending_evict.pop()
                            nc.vector.tensor_copy(
                                dst[:, 0:512], pj_ps[96:128, 0:512])
                            nc.scalar.copy(
                                dst[:, 512:1024], pj_ps[64:96, 512:1024])
                            nc.vector.tensor_copy(
                                dst[:, 1024:1536], pj_ps[0:32, 0:512])
                            nc.scalar.copy(
                                dst[:, 1536:2048], pj_ps[32:64, 512:1024])

                    for s in range(NSLOT):
                        flush_evicts()
                        ts_ = [s, s - LAG1, s - LAG2]
                        act = [0 <= t < t_steps for t in ts_]
                        rec = [act[l] and ts_[l] >= 1 for l in range(3)]
                        steady = all(act) and all(rec)
                        prefetch_xz0(s + 4)

                        # projection schedule: two steps of one layer/slot
                        pj = None
                        if s % 2 == 0:
                            tau = s - 4
                            if 0 <= tau and tau + 1 < t_steps:
                                pj = (1, tau)
                        else:
                            tau = s - 13
                            if 0 <= tau and tau + 1 < t_steps:
                                pj = (2, tau)

                        def inject(l, b):
                            # xz -> PSUM via PE (start of accumulation group)
                            t = ts_[l]
                            nsl = NSL[b]
                            if l == 0:
                                nc.tensor.matmul(
                                    z_ps[ROWS[0], nsl], id16[:, :],
                                    ring0[t % RING][:, nsl],
                                    start=True, stop=not rec[0],
                                    tile_position=(0, 0))
                            else:
                                src = pjring[l][(t // 2) % NPJR]
                                selm = sel_ev if t % 2 == 0 else sel_od
                                nc.tensor.matmul(
                                    z_ps[ROWS[l], nsl], selm[:, :],
                                    src[:, nsl],
                                    start=True, stop=not rec[l],
                                    tile_position=(0, 32 * l))

                        def pj_block():
                            # all 4 gate chunks on 4 distinct PE column
                            # lanes, k-rounds interleaved -> 4-way concurrent
                            L, tau = pj
                            r = (tau + LAGS[L - 1]) % HR
                            for k in range(5):
                                for b in range(4):
                                    rb, pc = PJP[b]
                                    if k < 4:
                                        nc.tensor.matmul(
                                            pj_ps[rb:rb + 32, pc],
                                            hTr[:, k, L - 1, r:r + 2, :],
                                            W_sb[L][:, k, NSL[b]],
                                            start=(k == 0), stop=False,
                                            tile_position=(0, rb))
                                    else:
                                        nc.tensor.matmul(
                                            pj_ps[rb:rb + 32, pc],
                                            ones1[:, :],
                                            biasrow[L][:, NSL[b]],
                                            start=False, stop=True,
                                            tile_position=(0, rb))

                        # per-slot work tiles (double-buffered via pool)
                        sig_f = wk.tile([80, H], f32, tag="sigf")
                        g_t = wk.tile([80, H], f32, tag="g")
                        sig_i = wk.tile([80, H], f32, tag="sigi")
                        sig_o = wk.tile([80, H], f32, tag="sigo")
                        ig = wk.tile([80, H], f32, tag="ig")
                        tch = wk.tile([80, H], f32, tag="tch")

                        def emit_gate(b):
                            if steady:
                                z80 = z_ps[0:80]
                                if b == 0:
                                    nc.scalar.activation(
                                        sig_f[:], z80[:, F_], AF.Sigmoid)
                                    nc.vector.tensor_tensor(
                                        c_sb[:], c_sb[:], sig_f[:], OP.mult)
                                elif b == 1:
                                    nc.scalar.activation(
                                        g_t[:], z80[:, G_], AF.Tanh)
                                elif b == 2:
                                    nc.scalar.activation(
                                        sig_i[:], z80[:, I_], AF.Sigmoid)
                                    for hh in (HA, HB):
                                        nc.vector.tensor_tensor(
                                            ig[:, hh], sig_i[:, hh],
                                            g_t[:, hh], OP.mult)
                                    for hh in (HA, HB):
                                        nc.vector.tensor_tensor(
                                            c_sb[:, hh], c_sb[:, hh],
                                            ig[:, hh], OP.add)
                                else:
                                    nc.scalar.activation(
                                        sig_o[:], z80[:, O_], AF.Sigmoid)
                                    for hh in (HA, HB):
                                        nc.scalar.activation(
                                            tch[:, hh], c_sb[:, hh], AF.Tanh)
                                        nc.vector.tensor_tensor(
                                            h_bf[0:80, hh], sig_o[:, hh],
                                            tch[:, hh], OP.mult)
                                return
                            for l in range(3):
                                if not act[l]:
                                    continue
                                r = ROWS[l]
                                if b == 0:
                                    nc.scalar.activation(
                                        sig_f[r, :], z_ps[r, F_], AF.Sigmoid)
                                    if rec[l]:
                                        nc.vector.tensor_tensor(
                                            c_sb[r, :], c_sb[r, :],
                                            sig_f[r, :], OP.mult)
                                elif b == 1:
                                    nc.scalar.activation(
                                        g_t[r, :], z_ps[r, G_], AF.Tanh)
                                elif b == 2:
                                    nc.scalar.activation(
                                        sig_i[r, :], z_ps[r, I_], AF.Sigmoid)
                                    nc.vector.tensor_tensor(
                                        ig[r, :], sig_i[r, :], g_t[r, :],
                                        OP.mult)
                                    if rec[l]:
                                        nc.vector.tensor_tensor(
                                            c_sb[r, :], c_sb[r, :],
                                            ig[r, :], OP.add)
                                    else:
                                        nc.vector.tensor_copy(
                                            c_sb[r, :], ig[r, :])
                                else:
                                    nc.scalar.activation(
                                        sig_o[r, :], z_ps[r, O_], AF.Sigmoid)
                                    nc.scalar.activation(
                                        tch[r, :], c_sb[r, :], AF.Tanh)
                                    nc.vector.tensor_tensor(
                                        h_bf[r, :], sig_o[r, :], tch[r, :],
                                        OP.mult)

                        # ---- matmuls: 4 gate banks x (inject + 4k) ----
                        for b in range(4):
                            nsl = NSL[b]
                            for l in range(3):
                                if act[l]:
                                    inject(l, b)
                            for k in range(4):
                                for l in range(3):
                                    if rec[l]:
                                        nc.tensor.matmul(
                                            z_ps[ROWS[l], nsl],
                                            hTr[:, k, l, (s - 1) % HR, :],
                                            U_sb[l][:, k, nsl],
                                            start=False, stop=(k == 3),
                                            tile_position=(0, 32 * l))
                            emit_gate(b)

                        # pj runs after the recurrent banks on 4 lanes
                        if pj is not None:
                            pj_block()

                        # ---- transpose h into hTr ring (PE + DVE copy) ----
                        if any(act):
                            for cch in range(4):
                                nc.tensor.transpose(
                                    tr_ps[:, cch, :],
                                    h_bf[0:96, 128 * cch:128 * (cch + 1)],
                                    id96[:, :])
                            nc.vector.tensor_copy(
                                hTr[:, :, :, s % HR, :],
                                tr_ps[:].rearrange(
                                    "p k (l c) -> p k l c", c=32)
                                [:, :, :, 0:BL])

                        # ---- pj evictions (PSUM -> staged xz ring) are
                        # deferred to the start of the next slot, where ACT
                        # and DVE are otherwise idle
                        if pj is not None:
                            L, tau = pj
                            dst = pjring[L][(tau // 2) % NPJR]
                            pending_evict.append((dst, pj))

                        if debug is True:
                            if act[0]:
                                nc.sync.dma_start(
                                    h0rows[ts_[0] * BL:(ts_[0] + 1) * BL, :],
                                    h_bf[0:BL, :])
                            if act[1]:
                                nc.sync.dma_start(
                                    h1rows[ts_[1] * BL:(ts_[1] + 1) * BL, :],
                                    h_bf[32:32 + BL, :])
                            if pj is not None:
                                Ld, taud = pj
                                if Ld == 1:
                                    dsts = pjring[1][(taud // 2) % NPJR]
                                    nc.gpsimd.dma_start(
                                        xz1rows[taud * BL:(taud + 2) * BL, :],
                                        dsts[:, :])
                        # ---- layer-2 outputs for attention ----
                        if act[2]:
                            t2 = ts_[2]
                            nc.sync.dma_start(
                                h2rows[t2 * BL:(t2 + 1) * BL, :],
                                h_bf[64:64 + BL, :])
                            nc.sync.dma_start(
                                h2T.rearrange("k p t b -> p k t b")
                                [:, :, t2, :],
                                hTr[:, :, 2, s % HR, :])

            xz_pass()
            wavefront()

            # ---------------- attention ----------------
            with ExitStack() as ctx:
                cst = ctx.enter_context(tc.tile_pool(name="atc", bufs=1))
                Wa_sb = cst.tile([P, 4, H], bf16, name="atWa")
                nc.sync.dma_start(
                    Wa_sb[:], Wab.rearrange("(k p) n -> p k n", p=P))
                ba_rep = cst.tile([P, H], f32)
                nc.sync.dma_start(ba_rep[:], ba[None, :].to_broadcast((P, H)))
                s_sb = cst.tile([P, MT], f32)
                io = ctx.enter_context(tc.tile_pool(name="atio", bufs=3))
                ps = ctx.enter_context(
                    tc.tile_pool(name="atps", bufs=2, space="PSUM"))
                # e-pass: s[(t,b)] = sum_k tanh(h2 @ Wa + ba)
                for m in range(MT):
                    kxm = io.tile([P, 4, TPB, BL], bf16, tag="kxm")
                    for k in range(4):
                        nc.sync.dma_start(
                            kxm[:, k],
                            h2T[k, :, m * TPB:(m + 1) * TPB, :])
                    ep = ps.tile([P, H], f32, tag="ep")
                    for k in range(4):
                        nc.tensor.matmul(
                            ep[:], kxm[:, k], Wa_sb[:, k, :],
                            start=(k == 0), stop=(k == 3))
                    e_sb = io.tile([P, H], f32, tag="e")
                    nc.vector.tensor_tensor(e_sb[:], ep[:], ba_rep[:], OP.add)
                    e_t = io.tile([P, H], f32, tag="et")
                    nc.scalar.activation(e_t[:], e_sb[:], AF.Tanh,
                                         accum_out=s_sb[:, m:m + 1])

                # s (row layout [P, MT]) -> sT [BL, t_steps] via flat DRAM
                nc.sync.dma_start(
                    s_dram.rearrange("(m p) -> p m", p=P), s_sb[:])
                sT = cst.tile([BL, t_steps], f32)
                nc.sync.dma_start(
                    sT[:], s_dram.rearrange("(t b) -> b t", b=BL))
                mx = cst.tile([BL, 1], f32)
                nc.vector.reduce_max(mx[:], sT[:], axis=mybir.AxisListType.X)
                nmx = cst.tile([BL, 1], f32)
                nc.vector.tensor_scalar_mul(nmx[:], mx[:], -1.0)
                ex = cst.tile([BL, t_steps], f32)
                sm = cst.tile([BL, 1], f32)
                nc.scalar.activation(ex[:], sT[:], AF.Exp, bias=nmx[:],
                                     accum_out=sm[:])
                rs = cst.tile([BL, 1], f32)
                nc.vector.reciprocal(rs[:], sm[:])
                aT = cst.tile([BL, t_steps], f32)
                nc.vector.tensor_scalar_mul(aT[:], ex[:], rs[:])
                nc.sync.dma_start(
                    a_dram.rearrange("(t b) -> b t", b=BL), aT[:])
                a_row = cst.tile([P, MT], f32)
                nc.sync.dma_start(
                    a_row[:], a_dram.rearrange("(m p) -> p m", p=P))

                # pooled[b, :] = sum_rows sel * (a * h2)   (f32 matmuls)
                pp = ctx.enter_context(
                    tc.tile_pool(name="atpp", bufs=1, space="PSUM"))
                ps1 = ctx.enter_context(
                    tc.tile_pool(name="atp1", bufs=1, space="PSUM"))
                pooled_ps = pp.tile([BL, H], f32)
                for m in range(MT):
                    h2t = io.tile([P, H], bf16, tag="h2t")
                    nc.sync.dma_start(h2t[:], h2rows[m * P:(m + 1) * P, :])
                    wrow = io.tile([P, H], f32, tag="wrow")
                    nc.vector.tensor_scalar_mul(wrow[:], h2t[:],
                                                a_row[:, m:m + 1])
                    nc.tensor.matmul(pooled_ps[:], sel[:], wrow[:],
                                     start=(m == 0), stop=(m == MT - 1))

                # pooledT via PE transpose
                pooled_sb = cst.tile([BL, H], f32)
                nc.vector.tensor_copy(pooled_sb[:], pooled_ps[:])
                ptp = ps1.tile([P, 4 * BL], f32, tag="ptp")
                for k in range(4):
                    nc.tensor.transpose(
                        ptp[:, k * BL:(k + 1) * BL],
                        pooled_sb[:, k * P:(k + 1) * P], ident[0:BL, 0:BL])
                pooledT = cst.tile([P, 4, BL], f32r)
                nc.vector.tensor_copy(
                    pooledT[:], ptp[:].rearrange("p (k b) -> p k b", k=4))

                # ---------------- dense head ----------------
                def load_r(pool, dram_ap, shape, name):
                    stg = pool.tile(shape, f32, name=name + "_stg")
                    nc.sync.dma_start(stg[:], dram_ap)
                    t_ = pool.tile(shape, f32r, name=name)
                    nc.any.tensor_copy(t_[:], stg[:])
                    return t_

                Wd1_sb = load_r(cst, Wd1p.rearrange("(k p) n -> p k n", p=P),
                                [P, 4, P], "hWd1")
                bd1_sb = cst.tile([P, 1], f32)
                nc.sync.dma_start(bd1_sb[:], bd1[:, None])
                Wd2_sb = load_r(cst, Wd2p[:, :], [P, 64], "hWd2")
                bd2_sb = cst.tile([64, 1], f32)
                nc.sync.dma_start(bd2_sb[:], bd2p[:, None])
                Wd3_sb = load_r(cst, Wd3[:, :], [64, 5], "hWd3")
                bd3_sb = cst.tile([5, 1], f32)
                nc.sync.dma_start(bd3_sb[:], bd3[:, None])

                d1p = ps1.tile([P, BL], f32, tag="d1p")
                for k in range(4):
                    nc.tensor.matmul(d1p[:], Wd1_sb[:, k, :], pooledT[:, k, :],
                                     start=(k == 0), stop=(k == 3))
                d1 = cst.tile([P, BL], f32r)
                nc.scalar.activation(d1[:], d1p[:], AF.Relu, bias=bd1_sb[:])
                d2p = ps1.tile([64, BL], f32, tag="d2p")
                nc.tensor.matmul(d2p[:], Wd2_sb[:], d1[:], start=True,
                                 stop=True)
                d2 = cst.tile([64, BL], f32r)
                nc.scalar.activation(d2[:], d2p[:], AF.Relu, bias=bd2_sb[:])
                d3p = ps1.tile([5, BL], f32, tag="d3p")
                nc.tensor.matmul(d3p[:], Wd3_sb[:], d2[:], start=True,
                                 stop=True)
                d3 = cst.tile([5, BL], f32)
                nc.scalar.activation(d3[:], d3p[:], AF.Identity, bias=bd3_sb[:])
                nc.sync.dma_start(outT[:, :], d3[:])

    nc.compile()
    return nc


@functools.lru_cache(maxsize=2)
def _compiled(t_steps):
    return build_nc(t_steps)


def _make_in_maps(inputs):
    w = prep_weights(inputs)
    x = np.ascontiguousarray(np.asarray(inputs['x'], np.float32))
    base = {k: w[k] for k in (
        'W0p', 'b0p', 'U0b', 'U1b', 'U2b', 'W1b', 'b1pb', 'W2b', 'b2pb',
        'Wab', 'ba', 'Wd1p', 'bd1', 'Wd2p', 'bd2p', 'Wd3', 'bd3', 'sel',
        'ident', 'id16b', 'sel_ev', 'sel_od', 'ones1', 'id96b')}
    in_maps = []
    for c in range(NC):
        m = dict(base)
        m['xT'] = np.ascontiguousarray(
            x[c * BL:(c + 1) * BL].transpose(2, 1, 0))
        in_maps.append(m)
    return in_maps


def kernel(**inputs):
    from concourse import bass_utils
    nc = _compiled(T)
    in_maps = _make_in_maps(inputs)
    res = bass_utils.run_bass_kernel_spmd(nc, in_maps, core_ids=list(range(NC)))
    out = np.concatenate([np.asarray(res.results[c]['outT']).T
                          for c in range(NC)], axis=0)
    return np.ascontiguousarray(out, np.float32)


def timed_run(tmpdir=None, **inputs):
    """Run with NTFF profiling; returns BassKernelResults."""
    from concourse import bass_utils
    nc = _compiled(T)
    in_maps = _make_in_maps(inputs)
    res = bass_utils.run_bass_kernel_spmd(
        nc, in_maps, core_ids=list(range(NC)), trace=True, tmpdir=tmpdir)
    return res
